# revision 36
# baseline (speedup 1.0000x reference)
"""GAT influence layer on 8 Trainium2 NeuronCores (Bass/Tile), fp16 edition.

Strategy (edge-parallel, row-sharded):
  Pass 1 (device): each core computes its 12.5k-node slice of
      [Wh | Wh@a_src | Wh@a_dst] = h @ [W | W@a_src | W@a_dst]
      as fp16 TensorE matmuls against a host-augmented weight matrix.
  Host: buckets edges by 32-node destination block, permutes blocks onto
      (core, slot) pairs balancing per-slot tile counts, and builds per-core
      fp16 message streams (Wh[col] rows + ones column), an f32 q stream
      (s_src[row]+s_dst[col], global-max handled via a baked exp bias) and an
      fp16 row-rel stream.  Data movement only.
  Pass 2 (device): ACT computes exp(leakyrelu(q) - emax) (fp16); DVE builds a
      per-superblock exp-weighted one-hot selection matrix in fp16 at 2x_1p
      rate (u-major/t-minor layout keeps every operand's last dim packed);
      TensorE does the softmax-weighted segment-sum as PSUM-accumulated fp16
      matmuls; denominators are batch-reciprocal'd on DVE and the final
      division rides the PSUM->SBUF copy (ACT scaled copies + a DVE batched
      tail).  Large DMAs alternate between the two HWDGE queues.
  Host: concatenates per-core node-partitioned fp16 outputs, casts to f32.
"""

import os
import numpy as np

N_NODES = 100000
N_EDGES = 1600000
IN_DIM = 128
OUT_DIM = 64
NEG_SLOPE = 0.2
CORES = 8
NPC = N_NODES // CORES          # nodes per core (12500)
BW = 32                         # max nodes per block (matmul window)
NPP = 12544                     # padded nodes per core, pass 1
W65 = OUT_DIM + 1
SBB = 17                        # blocks per superblock
PGRP = (7, 7, 3)                # psum group sizes (7*65=455 f32 cols per bank)
PAD_Q = -30000.0                # pad-slot attention logit -> exp == 0
DVE_TAIL = 6                    # trailing group-1 blocks divided on DVE
DP_LAM = 0.5                    # per-block tile-equivalent penalty in the DP

LAST_STATS = {}


def _build_pass1():
    from concourse import bacc, mybir
    import concourse.tile as tile

    f16 = mybir.dt.float16
    f32 = mybir.dt.float32
    act = mybir.ActivationFunctionType
    nc = bacc.Bacc("TRN2", target_bir_lowering=False, debug=False)
    d_hT = nc.dram_tensor("hT", [128, NPP], f16, kind="ExternalInput")
    d_waug = nc.dram_tensor("waug", [IN_DIM, W65 + 1], f16, kind="ExternalInput")
    d_whT = nc.dram_tensor("whT", [W65 + 1, NPP], f16, kind="ExternalOutput")

    NW = 512
    CHW = 6 * NW                # 3072-col chunks
    with tile.TileContext(nc) as tc:
        with tc.tile_pool(name="c1", bufs=1) as cp, \
             tc.tile_pool(name="ht1", bufs=5) as hp, \
             tc.tile_pool(name="wo1", bufs=3) as wo, \
             tc.tile_pool(name="ps1", bufs=6, space="PSUM") as psp:
            # all hT input DMAs issued upfront, split across both queues;
            # whT output DMAs trail behind them (gated on casts, they can
            # then never head-of-line-block an input load)
            waug = cp.tile([IN_DIM, W65 + 1], f16)
            nc.scalar.dma_start(out=waug[:], in_=d_waug[:])

            chunks = []
            for ci, g0 in enumerate(range(0, NPP, CHW)):
                g1 = min(g0 + CHW, NPP)
                ht = hp.tile([128, CHW], f16, tag="ht")
                eng = nc.sync if ci % 2 == 0 else nc.scalar
                eng.dma_start(out=ht[:, :g1 - g0], in_=d_hT[:, g0:g1])
                chunks.append((g0, g1, ht))
            for ci, (g0, g1, ht) in enumerate(chunks):
                gw = g1 - g0
                wh_sb = wo.tile([W65 + 1, CHW], f16, tag="wh")
                for ki, c0 in enumerate(range(0, gw, NW)):
                    w = min(c0 + NW, gw) - c0
                    wh_ps = psp.tile([W65 + 1, NW], f32, space="PSUM")
                    nc.tensor.matmul(out=wh_ps[:, :w], lhsT=waug[:],
                                     rhs=ht[:, c0:c0 + w], start=True, stop=True)
                    if ki % 2 == 0:
                        nc.vector.tensor_copy(out=wh_sb[:, c0:c0 + w],
                                              in_=wh_ps[:, :w])
                    else:
                        nc.scalar.activation(out=wh_sb[:, c0:c0 + w],
                                             in_=wh_ps[:, :w], func=act.Copy)
                eng2 = nc.scalar if ci % 2 == 0 else nc.sync
                eng2.dma_start(out=d_whT[:, g0:g1], in_=wh_sb[:, :gw])
    nc.compile()
    return nc


def _build_pass2(Tj, Ttot, emax, bpc):
    from concourse import bacc, mybir
    import concourse.tile as tile

    f16 = mybir.dt.float16
    f32 = mybir.dt.float32
    alu = mybir.AluOpType
    act = mybir.ActivationFunctionType

    nsb = bpc // SBB
    npp2 = bpc * BW
    base = np.zeros(bpc + 1, np.int64)
    base[1:] = np.cumsum(Tj)
    assert base[-1] == Ttot
    TMAX = int(max(base[(s + 1) * SBB] - base[s * SBB] for s in range(nsb)))

    nc = bacc.Bacc("TRN2", target_bir_lowering=False, debug=False)
    d_msg = nc.dram_tensor("msg", [128, Ttot * W65], f16, kind="ExternalInput")
    d_q = nc.dram_tensor("q", [128, Ttot], f32, kind="ExternalInput")
    d_rr = nc.dram_tensor("rr", [128, Ttot], f16, kind="ExternalInput")
    d_iota = nc.dram_tensor("iota", [128, BW * TMAX], f16, kind="ExternalInput")
    d_out = nc.dram_tensor("out", [npp2, OUT_DIM], f16, kind="ExternalOutput")

    # q chunk boundaries: first superblock alone, then 3 balanced chunks
    qcuts = [0, int(base[SBB])]
    rest = Ttot - qcuts[1]
    for k in range(3):
        qcuts.append(qcuts[1] + ((k + 1) * rest) // 3)

    with tile.TileContext(nc) as tc:
        with tc.tile_pool(name="c2", bufs=1) as cp, \
             tc.tile_pool(name="gp", bufs=4) as gp, \
             tc.tile_pool(name="mp", bufs=3) as mp, \
             tc.tile_pool(name="fp", bufs=4) as fp, \
             tc.tile_pool(name="op", bufs=3) as op, \
             tc.tile_pool(name="ppa", bufs=3, space="PSUM") as ppa, \
             tc.tile_pool(name="ppb", bufs=3, space="PSUM") as ppb, \
             tc.tile_pool(name="ppc", bufs=2, space="PSUM") as ppc:

            pools = (ppa, ppb, ppc)

            def sb_rng(s):
                j0 = s * SBB
                t0, t1 = int(base[j0]), int(base[j0 + SBB])
                return j0, t0, t1

            def load_g(s, split=False):
                j0, t0, t1 = sb_rng(s)
                eng_g = nc.sync if s % 2 == 0 else nc.scalar
                G = gp.tile([128, TMAX * W65], f16, tag="G", name="G")
                if split:
                    tc2 = int(base[j0 + 2])  # first 2 blocks land early
                    eng_g.dma_start(out=G[:, :(tc2 - t0) * W65],
                                    in_=d_msg[:, t0 * W65:tc2 * W65])
                    eng_g.dma_start(out=G[:, (tc2 - t0) * W65:(t1 - t0) * W65],
                                    in_=d_msg[:, tc2 * W65:t1 * W65])
                else:
                    eng_g.dma_start(out=G[:, :(t1 - t0) * W65],
                                    in_=d_msg[:, t0 * W65:t1 * W65])
                return G

            # sb0's G goes first on the sync queue; q/rr/iota ride scalar so
            # the first matmul's inputs land as early as possible
            G0 = load_g(0, split=True)

            q_sb = cp.tile([128, Ttot], f32)
            sc_sb = cp.tile([128, Ttot], f32)
            rr_sb = cp.tile([128, Ttot], f16)
            ex_sb = cp.tile([128, Ttot], f16)
            iota_rep = cp.tile([128, BW * TMAX], f16)
            # host pre-shifts q by -emax; leakyrelu(q)-emax == max(qA, sc)
            # with qA = q-emax, sc = 0.2*qA - 0.8*emax (HW Lrelu ignores alpha)
            for k in range(4):
                lo, hi = qcuts[k], qcuts[k + 1]
                eng = nc.scalar if k % 2 == 0 else nc.sync
                eng.dma_start(out=q_sb[:, lo:hi], in_=d_q[:, lo:hi])
                eng.dma_start(out=rr_sb[:, lo:hi], in_=d_rr[:, lo:hi])
                if k == 0:
                    # one-hot comparand, u-major/t-minor:
                    # iota_rep[p, u*TMAX + t] = u
                    nc.scalar.dma_start(out=iota_rep[:], in_=d_iota[:])
                nc.vector.tensor_scalar(out=sc_sb[:, lo:hi], in0=q_sb[:, lo:hi],
                                        scalar1=NEG_SLOPE,
                                        scalar2=-0.8 * float(emax),
                                        op0=alu.mult, op1=alu.add)
                nc.vector.tensor_tensor(out=q_sb[:, lo:hi], in0=q_sb[:, lo:hi],
                                        in1=sc_sb[:, lo:hi], op=alu.max)
                nc.scalar.activation(out=ex_sb[:, lo:hi], in_=q_sb[:, lo:hi],
                                     func=act.Exp)

            def build_m(s):
                _, t0, t1 = sb_rng(s)
                T_s = t1 - t0
                # M[p, u*T_s + t] = ex[p,t] * (u == rr[p,t]); packed fp16
                # last dims everywhere -> DVE 2x_1p
                M = mp.tile([128, BW * TMAX], f16, tag="M", name="M")
                M3 = M[:, :BW * T_s].rearrange("p (u t) -> p u t", u=BW)
                io3 = iota_rep[:].rearrange("p (u t) -> p u t", u=BW)[:, :, :T_s]
                rr3 = rr_sb[:, t0:t1].rearrange("p (o t) -> p o t", o=1) \
                                     .to_broadcast([128, BW, T_s])
                ex3 = ex_sb[:, t0:t1].rearrange("p (o t) -> p o t", o=1) \
                                     .to_broadcast([128, BW, T_s])
                nc.vector.tensor_tensor(out=M3, in0=io3, in1=rr3, op=alu.is_equal)
                nc.vector.tensor_tensor(out=M3, in0=M3, in1=ex3, op=alu.mult)
                return M

            # G and M both pipelined two superblocks ahead: out-DMA triggers
            # (gated on division) never head-of-line-block the next G load,
            # and each superblock's division+PSUM-release runs on DVE well
            # before the matmuls that reuse those PSUM banks
            Gq = [G0, load_g(1)]
            Mq = [build_m(0), build_m(1)]
            for s in range(nsb):
                j0, t0, t1 = sb_rng(s)
                T_s = t1 - t0
                eng_o = nc.scalar if s % 2 == 0 else nc.sync
                G = Gq.pop(0)
                M = Mq.pop(0)
                if s + 2 < nsb:
                    Gq.append(load_g(s + 2))
                    Mq.append(build_m(s + 2))

                pgs = []
                for g, gsz in enumerate(PGRP):
                    pgs.append(pools[g].tile([BW, gsz * W65], f32, space="PSUM",
                                             tag=f"pg{g}", name=f"pg{g}"))
                jloc = 0
                for g, gsz in enumerate(PGRP):
                    for b in range(gsz):
                        j = j0 + jloc
                        tj = int(Tj[j])
                        tb = int(base[j]) - t0
                        for t in range(tj):
                            rel = tb + t
                            nc.tensor.matmul(
                                out=pgs[g][:, b * W65:(b + 1) * W65],
                                lhsT=M[:, rel:rel + (BW - 1) * T_s + 1:T_s],
                                rhs=G[:, rel * W65:(rel + 1) * W65],
                                start=(t == 0), stop=(t == tj - 1))
                        jloc += 1

                # batched denominators, gathered on ACT (eps via Copy bias):
                # dn[u, jloc] <- pg[:, 64::65] + 1e-10
                dn = fp.tile([BW, SBB], f32, tag="dn")
                o = 0
                for g, gsz in enumerate(PGRP):
                    nc.scalar.activation(out=dn[:, o:o + gsz],
                                         in_=pgs[g][:, OUT_DIM::W65],
                                         func=act.Copy, bias=1e-10)
                    o += gsz
                dinv = fp.tile([BW, SBB], f32, tag="di")
                nc.vector.reciprocal(out=dinv[:], in_=dn[:])

                out_stage = op.tile([BW, SBB * OUT_DIM], f16, tag="ost")
                # ACT: scaled per-block copies for groups 0,1 minus a tail
                # that DVE handles as batched broadcast-mults (engine balance);
                # the final superblock goes all-DVE so the post-matmul tail is
                # short (nothing overlaps it anyway)
                tail = PGRP[1] if s == nsb - 1 else DVE_TAIL
                dve_parts = [(1, PGRP[1] - tail, tail), (2, 0, PGRP[2])]
                if s == nsb - 1:
                    dve_parts.insert(0, (0, 0, PGRP[0]))
                jloc = 0
                for g, gsz in enumerate(PGRP[:2]):
                    for b in range(gsz):
                        if s == nsb - 1 or (g == 1 and b >= gsz - tail):
                            break
                        nc.scalar.activation(
                            out=out_stage[:, jloc * OUT_DIM:(jloc + 1) * OUT_DIM],
                            in_=pgs[g][:, b * W65:b * W65 + OUT_DIM],
                            func=act.Copy, scale=dinv[:, jloc:jloc + 1])
                        jloc += 1
                for g, b0, bn in dve_parts:
                    if bn == 0:
                        continue
                    jb = b0 + sum(PGRP[:g])
                    nc.vector.tensor_tensor(
                        out=out_stage[:, jb * OUT_DIM:(jb + bn) * OUT_DIM]
                            .rearrange("p (b f) -> p b f", b=bn),
                        in0=pgs[g][:, b0 * W65:(b0 + bn) * W65]
                            .rearrange("p (b f) -> p b f", b=bn)[:, :, :OUT_DIM],
                        in1=dinv[:, jb:jb + bn].rearrange(
                            "p (b o) -> p b o", o=1).to_broadcast([BW, bn, OUT_DIM]),
                        op=alu.mult)

                out_ap = d_out[j0 * BW:(j0 + SBB) * BW, :].rearrange(
                    "(b p) f -> p b f", p=BW)
                in_ap = out_stage[:].rearrange("p (b f) -> p b f", b=SBB)
                eng_o.dma_start(out=out_ap, in_=in_ap)
    nc.compile()
    return nc


def _block_bounds(row):
    """DP over node boundaries: contiguous blocks of <= BW nodes minimizing
    sum of ceil(cnt/128) + DP_LAM per block (128-aligned edge counts)."""
    deg = np.bincount(row, minlength=N_NODES)
    pre = np.zeros(N_NODES + 1, np.int64)
    pre[1:] = np.cumsum(deg)
    prel = pre.tolist()
    INF = float("inf")
    f = [INF] * (N_NODES + 1)
    f[0] = 0.0
    choice = [0] * (N_NODES + 1)
    for n in range(1, N_NODES + 1):
        best = INF
        bk = 1
        pn = prel[n]
        for m in range(max(0, n - BW), n):
            c = f[m] + (pn - prel[m] + 127) // 128 + DP_LAM
            if c < best:
                best = c
                bk = n - m
        f[n] = best
        choice[n] = bk
    bounds = [N_NODES]
    n = N_NODES
    while n > 0:
        n -= choice[n]
        bounds.append(n)
    return np.array(bounds[::-1], np.int64), pre


def _prep_structure(row, col):
    """Variable-size dest-node blocks (<=32 nodes, ~128-aligned edge counts);
    permute blocks onto (core, slot) pairs so that blocks sharing a slot
    have similar edge counts; assign each edge a (partition p, tile t)."""
    bounds, pre = _block_bounds(row)
    nb = len(bounds) - 1
    cnt_real = pre[bounds[1:]] - pre[bounds[:-1]]
    bpc = -(-nb // CORES)
    bpc = -(-bpc // SBB) * SBB          # pad to a multiple of SBB
    NGB = CORES * bpc
    cnt = np.zeros(NGB, np.int64)
    cnt[:nb] = cnt_real
    sorted_ids = np.argsort(-cnt, kind="stable")
    blk_core = np.empty(NGB, np.int64)
    blk_slot = np.empty(NGB, np.int64)
    k = np.arange(NGB)
    blk_core[sorted_ids] = k % CORES
    blk_slot[sorted_ids] = k // CORES
    Tj = np.maximum(1, (cnt[sorted_ids[::CORES]] + 127) // 128)
    base = np.zeros(bpc + 1, np.int64)
    base[1:] = np.cumsum(Tj)
    Ttot = int(base[-1])

    gb = np.searchsorted(bounds, row, side="right") - 1
    key = blk_core[gb] * bpc + blk_slot[gb]
    kcnt = np.bincount(key, minlength=NGB)
    order = np.argsort(key, kind="stable")
    starts = np.zeros(NGB, np.int64)
    starts[1:] = np.cumsum(kcnt)[:-1]
    rank = np.arange(N_EDGES, dtype=np.int64) - np.repeat(starts, kcnt)
    key_s = key[order]
    core_s = key_s // bpc
    slot_s = key_s - core_s * bpc
    t_loc = rank >> 7
    p_s = rank & 127
    tglob = base[slot_s] + t_loc
    return dict(order=order, core_s=core_s, p_s=p_s, tglob=tglob,
                rel_s=(row[order] - bounds[gb[order]]), Tj=Tj, base=base,
                Ttot=Ttot, sorted_ids=sorted_ids, bounds=bounds, nb=nb,
                bpc=bpc)


def _run_spmd(nc, in_maps, trace=False):
    from concourse import bass_utils
    res = bass_utils.run_bass_kernel_spmd(
        nc, in_maps, core_ids=list(range(CORES)), trace=trace)
    return res


def kernel(h, row, col, W, a):
    trace = bool(os.environ.get("GAT_TRACE"))
    if trace:
        try:
            import ntff_shim
            ntff_shim.install()
        except Exception:
            trace = False

    h = np.ascontiguousarray(np.asarray(h, dtype=np.float32))
    W = np.ascontiguousarray(np.asarray(W, dtype=np.float32))
    a = np.ascontiguousarray(np.asarray(a, dtype=np.float32)).reshape(2 * OUT_DIM)
    row = np.asarray(row).astype(np.int64)
    col = np.asarray(col).astype(np.int64)

    # ---- pass 1: [Wh | s_src | s_dst], node-sharded, fp16 ----
    nc1 = _build_pass1()
    waug = np.concatenate(
        [W, (W @ a[:OUT_DIM])[:, None], (W @ a[OUT_DIM:])[:, None]],
        axis=1).astype(np.float16)
    in_maps1 = []
    for c in range(CORES):
        hpad = np.zeros((NPP, IN_DIM), np.float16)
        hpad[:NPC] = h[c * NPC:(c + 1) * NPC]
        in_maps1.append({"hT": np.ascontiguousarray(hpad.T), "waug": waug})
    res1 = _run_spmd(nc1, in_maps1, trace=trace)
    if trace:
        LAST_STATS["pass1_ns"] = res1.exec_time_ns

    WhA = np.ones((N_NODES, W65), np.float16)
    s_src = np.empty(N_NODES, np.float32)
    s_dst = np.empty(N_NODES, np.float32)
    for c in range(CORES):
        whT = res1.results[c]["whT"]
        WhA[c * NPC:(c + 1) * NPC, :OUT_DIM] = whT[:OUT_DIM, :NPC].T
        s_src[c * NPC:(c + 1) * NPC] = whT[OUT_DIM, :NPC]
        s_dst[c * NPC:(c + 1) * NPC] = whT[OUT_DIM + 1, :NPC]

    # ---- host: edge-slot structure + replicated-Wh message streams ----
    st = _prep_structure(row, col)
    Tj, Ttot = st["Tj"], st["Ttot"]
    cs, ps, tg = st["core_s"], st["p_s"], st["tglob"]
    row_s = row[st["order"]]
    col_s = col[st["order"]]

    msg = np.zeros((CORES, 128, Ttot, W65), np.float16)
    msg[cs, ps, tg] = WhA[col_s]
    q_edge = s_src[row_s] + s_dst[col_s]
    emax = float(np.max(np.maximum(q_edge, NEG_SLOPE * q_edge)))
    q = np.full((CORES, 128, Ttot), PAD_Q, np.float32)
    q[cs, ps, tg] = q_edge - emax          # device leakyrelu expects q-emax
    rr = np.zeros((CORES, 128, Ttot), np.float16)
    rr[cs, ps, tg] = st["rel_s"].astype(np.float16)

    # ---- pass 2: attention + segment sum ----
    bpc = st["bpc"]
    nc2 = _build_pass2(Tj, Ttot, emax, bpc)
    base = st["base"]
    nsb = bpc // SBB
    TMAX = int(max(base[(s + 1) * SBB] - base[s * SBB] for s in range(nsb)))
    iota_np = np.broadcast_to(
        np.repeat(np.arange(BW, dtype=np.float16), TMAX)[None, :],
        (128, BW * TMAX))
    iota_np = np.ascontiguousarray(iota_np)
    in_maps2 = [{"msg": msg[c].reshape(128, Ttot * W65),
                 "q": q[c], "rr": rr[c], "iota": iota_np}
                for c in range(CORES)]
    res2 = _run_spmd(nc2, in_maps2, trace=trace)
    if trace:
        LAST_STATS["pass2_ns"] = res2.exec_time_ns
        LAST_STATS["total_ns"] = (res1.exec_time_ns or 0) + (res2.exec_time_ns or 0)

    out = np.empty((N_NODES, OUT_DIM), np.float32)
    sorted_ids = st["sorted_ids"]
    bounds, nb = st["bounds"], st["nb"]
    for c in range(CORES):
        dev = res2.results[c]["out"]
        for j in range(bpc):
            g = int(sorted_ids[j * CORES + c])
            if g >= nb:
                continue
            n0, n1 = int(bounds[g]), int(bounds[g + 1])
            out[n0:n1] = dev[j * BW:j * BW + (n1 - n0)]
    return out


# revision 38
# speedup vs baseline: 1.1059x; 1.1059x over previous
"""GAT influence layer on 8 Trainium2 NeuronCores (Bass/Tile), fp16 edition.

Strategy (edge-parallel, row-sharded):
  Pass 1 (device): each core computes its 12.5k-node slice of
      [Wh | Wh@a_src | Wh@a_dst] = h @ [W | W@a_src | W@a_dst]
      as fp16 TensorE matmuls against a host-augmented weight matrix.
  Host: buckets edges by 32-node destination block, permutes blocks onto
      (core, slot) pairs balancing per-slot tile counts, and builds per-core
      fp16 message streams (Wh[col] rows + ones column), an f32 q stream
      (s_src[row]+s_dst[col], global-max handled via a baked exp bias) and an
      fp16 row-rel stream.  Data movement only.
  Pass 2 (device): ACT computes exp(leakyrelu(q) - emax) (fp16); DVE builds a
      per-superblock exp-weighted one-hot selection matrix in fp16 at 2x_1p
      rate (u-major/t-minor layout keeps every operand's last dim packed);
      TensorE does the softmax-weighted segment-sum as PSUM-accumulated fp16
      matmuls; denominators are batch-reciprocal'd on DVE and the final
      division rides the PSUM->SBUF copy (ACT scaled copies + a DVE batched
      tail).  Large DMAs alternate between the two HWDGE queues.
  Host: concatenates per-core node-partitioned fp16 outputs, casts to f32.
"""

import os
import numpy as np

N_NODES = 100000
N_EDGES = 1600000
IN_DIM = 128
OUT_DIM = 64
NEG_SLOPE = 0.2
CORES = 8
NPC = N_NODES // CORES          # nodes per core (12500)
BW = 32                         # max nodes per block (matmul window)
NPP = 12544                     # padded nodes per core, pass 1
W65 = OUT_DIM + 1
SBB = 17                        # blocks per superblock
PGRP = (7, 7, 3)                # psum group sizes (7*65=455 f32 cols per bank)
PAD_Q = -30000.0                # pad-slot attention logit -> exp == 0
DVE_TAIL = 5                    # trailing group-1 blocks divided on DVE
DP_LAM = 0.5                    # per-block tile-equivalent penalty in the DP

LAST_STATS = {}


def _build_pass1():
    from concourse import bacc, mybir
    import concourse.tile as tile

    f16 = mybir.dt.float16
    f32 = mybir.dt.float32
    act = mybir.ActivationFunctionType
    nc = bacc.Bacc("TRN2", target_bir_lowering=False, debug=False)
    d_hT = nc.dram_tensor("hT", [128, NPP], f16, kind="ExternalInput")
    d_waug = nc.dram_tensor("waug", [IN_DIM, W65 + 1], f16, kind="ExternalInput")
    d_whT = nc.dram_tensor("whT", [W65 + 1, NPP], f16, kind="ExternalOutput")

    NW = 512
    CHW = 6 * NW                # 3072-col chunks
    with tile.TileContext(nc) as tc:
        with tc.tile_pool(name="c1", bufs=1) as cp, \
             tc.tile_pool(name="ht1", bufs=5) as hp, \
             tc.tile_pool(name="wo1", bufs=3) as wo, \
             tc.tile_pool(name="ps1", bufs=6, space="PSUM") as psp:
            # all hT input DMAs issued upfront, split across both queues;
            # whT output DMAs trail behind them (gated on casts, they can
            # then never head-of-line-block an input load)
            waug = cp.tile([IN_DIM, W65 + 1], f16)
            nc.scalar.dma_start(out=waug[:], in_=d_waug[:])

            chunks = []
            for ci, g0 in enumerate(range(0, NPP, CHW)):
                g1 = min(g0 + CHW, NPP)
                ht = hp.tile([128, CHW], f16, tag="ht")
                eng = nc.sync if ci % 2 == 0 else nc.scalar
                eng.dma_start(out=ht[:, :g1 - g0], in_=d_hT[:, g0:g1])
                chunks.append((g0, g1, ht))
            for ci, (g0, g1, ht) in enumerate(chunks):
                gw = g1 - g0
                wh_sb = wo.tile([W65 + 1, CHW], f16, tag="wh")
                for ki, c0 in enumerate(range(0, gw, NW)):
                    w = min(c0 + NW, gw) - c0
                    wh_ps = psp.tile([W65 + 1, NW], f32, space="PSUM")
                    nc.tensor.matmul(out=wh_ps[:, :w], lhsT=waug[:],
                                     rhs=ht[:, c0:c0 + w], start=True, stop=True)
                    if ki % 2 == 0:
                        nc.vector.tensor_copy(out=wh_sb[:, c0:c0 + w],
                                              in_=wh_ps[:, :w])
                    else:
                        nc.scalar.activation(out=wh_sb[:, c0:c0 + w],
                                             in_=wh_ps[:, :w], func=act.Copy)
                eng2 = nc.scalar if ci % 2 == 0 else nc.sync
                eng2.dma_start(out=d_whT[:, g0:g1], in_=wh_sb[:, :gw])
    nc.compile()
    return nc


def _build_pass2(Tj, Ttot, emax, bpc):
    from concourse import bacc, mybir
    import concourse.tile as tile

    f16 = mybir.dt.float16
    f32 = mybir.dt.float32
    alu = mybir.AluOpType
    act = mybir.ActivationFunctionType

    nsb = bpc // SBB
    npp2 = bpc * BW
    base = np.zeros(bpc + 1, np.int64)
    base[1:] = np.cumsum(Tj)
    assert base[-1] == Ttot
    TMAX = int(max(base[(s + 1) * SBB] - base[s * SBB] for s in range(nsb)))

    nc = bacc.Bacc("TRN2", target_bir_lowering=False, debug=False)
    d_msg = nc.dram_tensor("msg", [128, Ttot * W65], f16, kind="ExternalInput")
    d_q = nc.dram_tensor("q", [128, Ttot], f32, kind="ExternalInput")
    d_rr = nc.dram_tensor("rr", [128, Ttot], f16, kind="ExternalInput")
    d_iota = nc.dram_tensor("iota", [128, BW * TMAX], f16, kind="ExternalInput")
    d_out = nc.dram_tensor("out", [npp2, OUT_DIM], f16, kind="ExternalOutput")

    # q chunk boundaries: first superblock alone, then 3 balanced chunks
    qcuts = [0, int(base[SBB])]
    rest = Ttot - qcuts[1]
    for k in range(3):
        qcuts.append(qcuts[1] + ((k + 1) * rest) // 3)

    with tile.TileContext(nc) as tc:
        with tc.tile_pool(name="c2", bufs=1) as cp, \
             tc.tile_pool(name="gp", bufs=4) as gp, \
             tc.tile_pool(name="mp", bufs=3) as mp, \
             tc.tile_pool(name="fp", bufs=4) as fp, \
             tc.tile_pool(name="op", bufs=3) as op, \
             tc.tile_pool(name="ppa", bufs=3, space="PSUM") as ppa, \
             tc.tile_pool(name="ppb", bufs=3, space="PSUM") as ppb, \
             tc.tile_pool(name="ppc", bufs=2, space="PSUM") as ppc:

            pools = (ppa, ppb, ppc)

            def sb_rng(s):
                j0 = s * SBB
                t0, t1 = int(base[j0]), int(base[j0 + SBB])
                return j0, t0, t1

            def load_g(s, split=False):
                j0, t0, t1 = sb_rng(s)
                eng_g = nc.sync if s % 2 == 0 else nc.scalar
                G = gp.tile([128, TMAX * W65], f16, tag="G", name="G")
                if split:
                    tc2 = int(base[j0 + 2])  # first 2 blocks land early
                    eng_g.dma_start(out=G[:, :(tc2 - t0) * W65],
                                    in_=d_msg[:, t0 * W65:tc2 * W65])
                    eng_g.dma_start(out=G[:, (tc2 - t0) * W65:(t1 - t0) * W65],
                                    in_=d_msg[:, tc2 * W65:t1 * W65])
                else:
                    eng_g.dma_start(out=G[:, :(t1 - t0) * W65],
                                    in_=d_msg[:, t0 * W65:t1 * W65])
                return G

            # sb0's G goes first on the sync queue; q/rr/iota ride scalar so
            # the first matmul's inputs land as early as possible
            G0 = load_g(0, split=True)

            q_sb = cp.tile([128, Ttot], f32)
            sc_sb = cp.tile([128, Ttot], f32)
            rr_sb = cp.tile([128, Ttot], f16)
            ex_sb = cp.tile([128, Ttot], f16)
            iota_rep = cp.tile([128, BW * TMAX], f16)
            # host pre-shifts q by -emax; leakyrelu(q)-emax == max(qA, sc)
            # with qA = q-emax, sc = 0.2*qA - 0.8*emax (HW Lrelu ignores alpha)
            for k in range(4):
                lo, hi = qcuts[k], qcuts[k + 1]
                eng = nc.scalar if k % 2 == 0 else nc.sync
                eng.dma_start(out=q_sb[:, lo:hi], in_=d_q[:, lo:hi])
                eng.dma_start(out=rr_sb[:, lo:hi], in_=d_rr[:, lo:hi])
                if k == 0:
                    # one-hot comparand, u-major/t-minor:
                    # iota_rep[p, u*TMAX + t] = u
                    nc.scalar.dma_start(out=iota_rep[:], in_=d_iota[:])
                nc.vector.tensor_scalar(out=sc_sb[:, lo:hi], in0=q_sb[:, lo:hi],
                                        scalar1=NEG_SLOPE,
                                        scalar2=-0.8 * float(emax),
                                        op0=alu.mult, op1=alu.add)
                nc.vector.tensor_tensor(out=q_sb[:, lo:hi], in0=q_sb[:, lo:hi],
                                        in1=sc_sb[:, lo:hi], op=alu.max)
                nc.scalar.activation(out=ex_sb[:, lo:hi], in_=q_sb[:, lo:hi],
                                     func=act.Exp)

            def build_m(s):
                _, t0, t1 = sb_rng(s)
                T_s = t1 - t0
                # M[p, u*T_s + t] = ex[p,t] * (u == rr[p,t]); packed fp16
                # last dims everywhere -> DVE 2x_1p
                M = mp.tile([128, BW * TMAX], f16, tag="M", name="M")
                M3 = M[:, :BW * T_s].rearrange("p (u t) -> p u t", u=BW)
                io3 = iota_rep[:].rearrange("p (u t) -> p u t", u=BW)[:, :, :T_s]
                rr3 = rr_sb[:, t0:t1].rearrange("p (o t) -> p o t", o=1) \
                                     .to_broadcast([128, BW, T_s])
                ex3 = ex_sb[:, t0:t1].rearrange("p (o t) -> p o t", o=1) \
                                     .to_broadcast([128, BW, T_s])
                nc.vector.tensor_tensor(out=M3, in0=io3, in1=rr3, op=alu.is_equal)
                nc.vector.tensor_tensor(out=M3, in0=M3, in1=ex3, op=alu.mult)
                return M

            # G and M both pipelined two superblocks ahead: out-DMA triggers
            # (gated on division) never head-of-line-block the next G load,
            # and each superblock's division+PSUM-release runs on DVE well
            # before the matmuls that reuse those PSUM banks
            Gq = [G0, load_g(1)]
            Mq = [build_m(0), build_m(1)]
            for s in range(nsb):
                j0, t0, t1 = sb_rng(s)
                T_s = t1 - t0
                eng_o = nc.scalar if s % 2 == 0 else nc.sync
                G = Gq.pop(0)
                M = Mq.pop(0)
                if s + 2 < nsb:
                    Gq.append(load_g(s + 2))
                    Mq.append(build_m(s + 2))

                pgs = []
                for g, gsz in enumerate(PGRP):
                    pgs.append(pools[g].tile([BW, gsz * W65], f32, space="PSUM",
                                             tag=f"pg{g}", name=f"pg{g}"))
                jloc = 0
                for g, gsz in enumerate(PGRP):
                    for b in range(gsz):
                        j = j0 + jloc
                        tj = int(Tj[j])
                        tb = int(base[j]) - t0
                        for t in range(tj):
                            rel = tb + t
                            nc.tensor.matmul(
                                out=pgs[g][:, b * W65:(b + 1) * W65],
                                lhsT=M[:, rel:rel + (BW - 1) * T_s + 1:T_s],
                                rhs=G[:, rel * W65:(rel + 1) * W65],
                                start=(t == 0), stop=(t == tj - 1))
                        jloc += 1

                # batched denominators (eps folded into the gather):
                # dn[u, jloc] <- pg[:, 64::65] + 1e-10
                dn = fp.tile([BW, SBB], f32, tag="dn")
                o = 0
                for g, gsz in enumerate(PGRP):
                    nc.vector.tensor_scalar(out=dn[:, o:o + gsz],
                                            in0=pgs[g][:, OUT_DIM::W65],
                                            scalar1=1e-10, scalar2=None,
                                            op0=alu.add)
                    o += gsz
                dinv = fp.tile([BW, SBB], f32, tag="di")
                nc.vector.reciprocal(out=dinv[:], in_=dn[:])

                out_stage = op.tile([BW, SBB * OUT_DIM], f16, tag="ost")
                # ACT: scaled per-block copies for groups 0,1 minus a tail
                # that DVE handles as batched broadcast-mults (engine balance);
                # the final superblock goes all-DVE so the post-matmul tail is
                # short (nothing overlaps it anyway)
                tail = PGRP[1] if s == nsb - 1 else DVE_TAIL
                dve_parts = [(1, PGRP[1] - tail, tail), (2, 0, PGRP[2])]
                if s == nsb - 1:
                    dve_parts.insert(0, (0, 0, PGRP[0]))
                jloc = 0
                for g, gsz in enumerate(PGRP[:2]):
                    for b in range(gsz):
                        if s == nsb - 1 or (g == 1 and b >= gsz - tail):
                            break
                        nc.scalar.activation(
                            out=out_stage[:, jloc * OUT_DIM:(jloc + 1) * OUT_DIM],
                            in_=pgs[g][:, b * W65:b * W65 + OUT_DIM],
                            func=act.Copy, scale=dinv[:, jloc:jloc + 1])
                        jloc += 1
                for g, b0, bn in dve_parts:
                    if bn == 0:
                        continue
                    jb = b0 + sum(PGRP[:g])
                    nc.vector.tensor_tensor(
                        out=out_stage[:, jb * OUT_DIM:(jb + bn) * OUT_DIM]
                            .rearrange("p (b f) -> p b f", b=bn),
                        in0=pgs[g][:, b0 * W65:(b0 + bn) * W65]
                            .rearrange("p (b f) -> p b f", b=bn)[:, :, :OUT_DIM],
                        in1=dinv[:, jb:jb + bn].rearrange(
                            "p (b o) -> p b o", o=1).to_broadcast([BW, bn, OUT_DIM]),
                        op=alu.mult)

                out_ap = d_out[j0 * BW:(j0 + SBB) * BW, :].rearrange(
                    "(b p) f -> p b f", p=BW)
                in_ap = out_stage[:].rearrange("p (b f) -> p b f", b=SBB)
                eng_o.dma_start(out=out_ap, in_=in_ap)
    nc.compile()
    return nc


def _block_bounds(row):
    """DP over node boundaries: contiguous blocks of <= BW nodes minimizing
    sum of ceil(cnt/128) + DP_LAM per block (128-aligned edge counts)."""
    deg = np.bincount(row, minlength=N_NODES)
    pre = np.zeros(N_NODES + 1, np.int64)
    pre[1:] = np.cumsum(deg)
    prel = pre.tolist()
    INF = float("inf")
    f = [INF] * (N_NODES + 1)
    f[0] = 0.0
    choice = [0] * (N_NODES + 1)
    for n in range(1, N_NODES + 1):
        best = INF
        bk = 1
        pn = prel[n]
        for m in range(max(0, n - BW), n):
            c = f[m] + (pn - prel[m] + 127) // 128 + DP_LAM
            if c < best:
                best = c
                bk = n - m
        f[n] = best
        choice[n] = bk
    bounds = [N_NODES]
    n = N_NODES
    while n > 0:
        n -= choice[n]
        bounds.append(n)
    return np.array(bounds[::-1], np.int64), pre


def _prep_structure(row, col):
    """Variable-size dest-node blocks (<=32 nodes, ~128-aligned edge counts);
    permute blocks onto (core, slot) pairs so that blocks sharing a slot
    have similar edge counts; assign each edge a (partition p, tile t)."""
    bounds, pre = _block_bounds(row)
    nb = len(bounds) - 1
    cnt_real = pre[bounds[1:]] - pre[bounds[:-1]]
    bpc = -(-nb // CORES)
    bpc = -(-bpc // SBB) * SBB          # pad to a multiple of SBB
    NGB = CORES * bpc
    cnt = np.zeros(NGB, np.int64)
    cnt[:nb] = cnt_real
    sorted_ids = np.argsort(-cnt, kind="stable")
    blk_core = np.empty(NGB, np.int64)
    blk_slot = np.empty(NGB, np.int64)
    k = np.arange(NGB)
    blk_core[sorted_ids] = k % CORES
    blk_slot[sorted_ids] = k // CORES
    Tj = np.maximum(1, (cnt[sorted_ids[::CORES]] + 127) // 128)
    base = np.zeros(bpc + 1, np.int64)
    base[1:] = np.cumsum(Tj)
    Ttot = int(base[-1])

    gb = np.searchsorted(bounds, row, side="right") - 1
    key = blk_core[gb] * bpc + blk_slot[gb]
    kcnt = np.bincount(key, minlength=NGB)
    order = np.argsort(key, kind="stable")
    starts = np.zeros(NGB, np.int64)
    starts[1:] = np.cumsum(kcnt)[:-1]
    rank = np.arange(N_EDGES, dtype=np.int64) - np.repeat(starts, kcnt)
    key_s = key[order]
    core_s = key_s // bpc
    slot_s = key_s - core_s * bpc
    t_loc = rank >> 7
    p_s = rank & 127
    tglob = base[slot_s] + t_loc
    return dict(order=order, core_s=core_s, p_s=p_s, tglob=tglob,
                rel_s=(row[order] - bounds[gb[order]]), Tj=Tj, base=base,
                Ttot=Ttot, sorted_ids=sorted_ids, bounds=bounds, nb=nb,
                bpc=bpc)


def _run_spmd(nc, in_maps, trace=False):
    from concourse import bass_utils
    res = bass_utils.run_bass_kernel_spmd(
        nc, in_maps, core_ids=list(range(CORES)), trace=trace)
    return res


def kernel(h, row, col, W, a):
    trace = bool(os.environ.get("GAT_TRACE"))
    if trace:
        try:
            import ntff_shim
            ntff_shim.install()
        except Exception:
            trace = False

    h = np.ascontiguousarray(np.asarray(h, dtype=np.float32))
    W = np.ascontiguousarray(np.asarray(W, dtype=np.float32))
    a = np.ascontiguousarray(np.asarray(a, dtype=np.float32)).reshape(2 * OUT_DIM)
    row = np.asarray(row).astype(np.int64)
    col = np.asarray(col).astype(np.int64)

    # ---- pass 1: [Wh | s_src | s_dst], node-sharded, fp16 ----
    nc1 = _build_pass1()
    waug = np.concatenate(
        [W, (W @ a[:OUT_DIM])[:, None], (W @ a[OUT_DIM:])[:, None]],
        axis=1).astype(np.float16)
    in_maps1 = []
    for c in range(CORES):
        hpad = np.zeros((NPP, IN_DIM), np.float16)
        hpad[:NPC] = h[c * NPC:(c + 1) * NPC]
        in_maps1.append({"hT": np.ascontiguousarray(hpad.T), "waug": waug})
    res1 = _run_spmd(nc1, in_maps1, trace=trace)
    if trace:
        LAST_STATS["pass1_ns"] = res1.exec_time_ns

    WhA = np.ones((N_NODES, W65), np.float16)
    s_src = np.empty(N_NODES, np.float32)
    s_dst = np.empty(N_NODES, np.float32)
    for c in range(CORES):
        whT = res1.results[c]["whT"]
        WhA[c * NPC:(c + 1) * NPC, :OUT_DIM] = whT[:OUT_DIM, :NPC].T
        s_src[c * NPC:(c + 1) * NPC] = whT[OUT_DIM, :NPC]
        s_dst[c * NPC:(c + 1) * NPC] = whT[OUT_DIM + 1, :NPC]

    # ---- host: edge-slot structure + replicated-Wh message streams ----
    st = _prep_structure(row, col)
    Tj, Ttot = st["Tj"], st["Ttot"]
    cs, ps, tg = st["core_s"], st["p_s"], st["tglob"]
    row_s = row[st["order"]]
    col_s = col[st["order"]]

    msg = np.zeros((CORES, 128, Ttot, W65), np.float16)
    msg[cs, ps, tg] = WhA[col_s]
    q_edge = s_src[row_s] + s_dst[col_s]
    emax = float(np.max(np.maximum(q_edge, NEG_SLOPE * q_edge)))
    q = np.full((CORES, 128, Ttot), PAD_Q, np.float32)
    q[cs, ps, tg] = q_edge - emax          # device leakyrelu expects q-emax
    rr = np.zeros((CORES, 128, Ttot), np.float16)
    rr[cs, ps, tg] = st["rel_s"].astype(np.float16)

    # ---- pass 2: attention + segment sum ----
    bpc = st["bpc"]
    nc2 = _build_pass2(Tj, Ttot, emax, bpc)
    base = st["base"]
    nsb = bpc // SBB
    TMAX = int(max(base[(s + 1) * SBB] - base[s * SBB] for s in range(nsb)))
    iota_np = np.broadcast_to(
        np.repeat(np.arange(BW, dtype=np.float16), TMAX)[None, :],
        (128, BW * TMAX))
    iota_np = np.ascontiguousarray(iota_np)
    in_maps2 = [{"msg": msg[c].reshape(128, Ttot * W65),
                 "q": q[c], "rr": rr[c], "iota": iota_np}
                for c in range(CORES)]
    res2 = _run_spmd(nc2, in_maps2, trace=trace)
    if trace:
        LAST_STATS["pass2_ns"] = res2.exec_time_ns
        LAST_STATS["total_ns"] = (res1.exec_time_ns or 0) + (res2.exec_time_ns or 0)

    out = np.empty((N_NODES, OUT_DIM), np.float32)
    sorted_ids = st["sorted_ids"]
    bounds, nb = st["bounds"], st["nb"]
    for c in range(CORES):
        dev = res2.results[c]["out"]
        for j in range(bpc):
            g = int(sorted_ids[j * CORES + c])
            if g >= nb:
                continue
            n0, n1 = int(bounds[g]), int(bounds[g + 1])
            out[n0:n1] = dev[j * BW:j * BW + (n1 - n0)]
    return out


# revision 39
# speedup vs baseline: 1.1154x; 1.0086x over previous
"""GAT influence layer on 8 Trainium2 NeuronCores (Bass/Tile), fp16 edition.

Strategy (edge-parallel, row-sharded):
  Pass 1 (device): each core computes its 12.5k-node slice of
      [Wh | Wh@a_src | Wh@a_dst] = h @ [W | W@a_src | W@a_dst]
      as fp16 TensorE matmuls against a host-augmented weight matrix.
  Host: buckets edges by 32-node destination block, permutes blocks onto
      (core, slot) pairs balancing per-slot tile counts, and builds per-core
      fp16 message streams (Wh[col] rows + ones column), an f32 q stream
      (s_src[row]+s_dst[col], global-max handled via a baked exp bias) and an
      fp16 row-rel stream.  Data movement only.
  Pass 2 (device): ACT computes exp(leakyrelu(q) - emax) (fp16); DVE builds a
      per-superblock exp-weighted one-hot selection matrix in fp16 at 2x_1p
      rate (u-major/t-minor layout keeps every operand's last dim packed);
      TensorE does the softmax-weighted segment-sum as PSUM-accumulated fp16
      matmuls; denominators are batch-reciprocal'd on DVE and the final
      division rides the PSUM->SBUF copy (ACT scaled copies + a DVE batched
      tail).  Large DMAs alternate between the two HWDGE queues.
  Host: concatenates per-core node-partitioned fp16 outputs, casts to f32.
"""

import os
import numpy as np

N_NODES = 100000
N_EDGES = 1600000
IN_DIM = 128
OUT_DIM = 64
NEG_SLOPE = 0.2
CORES = 8
NPC = N_NODES // CORES          # nodes per core (12500)
BW = 32                         # max nodes per block (matmul window)
NPP = 12544                     # padded nodes per core, pass 1
W65 = OUT_DIM + 1
SBB = 17                        # blocks per superblock
PGRP = (7, 7, 3)                # psum group sizes (7*65=455 f32 cols per bank)
PAD_Q = -30000.0                # pad-slot attention logit -> exp == 0
DVE_TAIL = 4                    # trailing group-1 blocks divided on DVE
DP_LAM = 0.5                    # per-block tile-equivalent penalty in the DP

LAST_STATS = {}


def _build_pass1():
    from concourse import bacc, mybir
    import concourse.tile as tile

    f16 = mybir.dt.float16
    f32 = mybir.dt.float32
    act = mybir.ActivationFunctionType
    nc = bacc.Bacc("TRN2", target_bir_lowering=False, debug=False)
    d_hT = nc.dram_tensor("hT", [128, NPP], f16, kind="ExternalInput")
    d_waug = nc.dram_tensor("waug", [IN_DIM, W65 + 1], f16, kind="ExternalInput")
    d_whT = nc.dram_tensor("whT", [W65 + 1, NPP], f16, kind="ExternalOutput")

    NW = 512
    CHW = 6 * NW                # 3072-col chunks
    with tile.TileContext(nc) as tc:
        with tc.tile_pool(name="c1", bufs=1) as cp, \
             tc.tile_pool(name="ht1", bufs=5) as hp, \
             tc.tile_pool(name="wo1", bufs=3) as wo, \
             tc.tile_pool(name="ps1", bufs=6, space="PSUM") as psp:
            # all hT input DMAs issued upfront, split across both queues;
            # whT output DMAs trail behind them (gated on casts, they can
            # then never head-of-line-block an input load)
            waug = cp.tile([IN_DIM, W65 + 1], f16)
            nc.scalar.dma_start(out=waug[:], in_=d_waug[:])

            chunks = []
            for ci, g0 in enumerate(range(0, NPP, CHW)):
                g1 = min(g0 + CHW, NPP)
                ht = hp.tile([128, CHW], f16, tag="ht")
                eng = nc.sync if ci % 2 == 0 else nc.scalar
                eng.dma_start(out=ht[:, :g1 - g0], in_=d_hT[:, g0:g1])
                chunks.append((g0, g1, ht))
            for ci, (g0, g1, ht) in enumerate(chunks):
                gw = g1 - g0
                wh_sb = wo.tile([W65 + 1, CHW], f16, tag="wh")
                for ki, c0 in enumerate(range(0, gw, NW)):
                    w = min(c0 + NW, gw) - c0
                    wh_ps = psp.tile([W65 + 1, NW], f32, space="PSUM")
                    nc.tensor.matmul(out=wh_ps[:, :w], lhsT=waug[:],
                                     rhs=ht[:, c0:c0 + w], start=True, stop=True)
                    if ki % 2 == 0:
                        nc.vector.tensor_copy(out=wh_sb[:, c0:c0 + w],
                                              in_=wh_ps[:, :w])
                    else:
                        nc.scalar.activation(out=wh_sb[:, c0:c0 + w],
                                             in_=wh_ps[:, :w], func=act.Copy)
                eng2 = nc.scalar if ci % 2 == 0 else nc.sync
                eng2.dma_start(out=d_whT[:, g0:g1], in_=wh_sb[:, :gw])
    nc.compile()
    return nc


def _build_pass2(Tj, Ttot, emax, bpc):
    from concourse import bacc, mybir
    import concourse.tile as tile

    f16 = mybir.dt.float16
    f32 = mybir.dt.float32
    alu = mybir.AluOpType
    act = mybir.ActivationFunctionType

    nsb = bpc // SBB
    npp2 = bpc * BW
    base = np.zeros(bpc + 1, np.int64)
    base[1:] = np.cumsum(Tj)
    assert base[-1] == Ttot
    TMAX = int(max(base[(s + 1) * SBB] - base[s * SBB] for s in range(nsb)))

    nc = bacc.Bacc("TRN2", target_bir_lowering=False, debug=False)
    d_msg = nc.dram_tensor("msg", [128, Ttot * W65], f16, kind="ExternalInput")
    d_q = nc.dram_tensor("q", [128, Ttot], f32, kind="ExternalInput")
    d_rr = nc.dram_tensor("rr", [128, Ttot], f16, kind="ExternalInput")
    d_iota = nc.dram_tensor("iota", [128, BW * TMAX], f16, kind="ExternalInput")
    d_out = nc.dram_tensor("out", [npp2, OUT_DIM], f16, kind="ExternalOutput")

    # q chunk boundaries: first superblock alone, then 3 balanced chunks
    qcuts = [0, int(base[SBB])]
    rest = Ttot - qcuts[1]
    for k in range(3):
        qcuts.append(qcuts[1] + ((k + 1) * rest) // 3)

    with tile.TileContext(nc) as tc:
        with tc.tile_pool(name="c2", bufs=1) as cp, \
             tc.tile_pool(name="gp", bufs=4) as gp, \
             tc.tile_pool(name="mp", bufs=3) as mp, \
             tc.tile_pool(name="fp", bufs=4) as fp, \
             tc.tile_pool(name="op", bufs=3) as op, \
             tc.tile_pool(name="ppa", bufs=3, space="PSUM") as ppa, \
             tc.tile_pool(name="ppb", bufs=3, space="PSUM") as ppb, \
             tc.tile_pool(name="ppc", bufs=2, space="PSUM") as ppc:

            pools = (ppa, ppb, ppc)

            def sb_rng(s):
                j0 = s * SBB
                t0, t1 = int(base[j0]), int(base[j0 + SBB])
                return j0, t0, t1

            def load_g(s, split=False):
                j0, t0, t1 = sb_rng(s)
                eng_g = nc.sync if s % 2 == 0 else nc.scalar
                G = gp.tile([128, TMAX * W65], f16, tag="G", name="G")
                if split:
                    tc2 = int(base[j0 + 2])  # first 2 blocks land early
                    eng_g.dma_start(out=G[:, :(tc2 - t0) * W65],
                                    in_=d_msg[:, t0 * W65:tc2 * W65])
                    eng_g.dma_start(out=G[:, (tc2 - t0) * W65:(t1 - t0) * W65],
                                    in_=d_msg[:, tc2 * W65:t1 * W65])
                else:
                    eng_g.dma_start(out=G[:, :(t1 - t0) * W65],
                                    in_=d_msg[:, t0 * W65:t1 * W65])
                return G

            # sb0's G goes first on the sync queue; q/rr/iota ride scalar so
            # the first matmul's inputs land as early as possible
            G0 = load_g(0, split=True)

            q_sb = cp.tile([128, Ttot], f32)
            sc_sb = cp.tile([128, Ttot], f32)
            rr_sb = cp.tile([128, Ttot], f16)
            ex_sb = cp.tile([128, Ttot], f16)
            iota_rep = cp.tile([128, BW * TMAX], f16)
            # host pre-shifts q by -emax; leakyrelu(q)-emax == max(qA, sc)
            # with qA = q-emax, sc = 0.2*qA - 0.8*emax (HW Lrelu ignores alpha)
            for k in range(4):
                lo, hi = qcuts[k], qcuts[k + 1]
                eng = nc.scalar if k % 2 == 0 else nc.sync
                eng.dma_start(out=q_sb[:, lo:hi], in_=d_q[:, lo:hi])
                eng.dma_start(out=rr_sb[:, lo:hi], in_=d_rr[:, lo:hi])
                if k == 0:
                    # one-hot comparand, u-major/t-minor:
                    # iota_rep[p, u*TMAX + t] = u
                    nc.scalar.dma_start(out=iota_rep[:], in_=d_iota[:])
                nc.vector.tensor_scalar(out=sc_sb[:, lo:hi], in0=q_sb[:, lo:hi],
                                        scalar1=NEG_SLOPE,
                                        scalar2=-0.8 * float(emax),
                                        op0=alu.mult, op1=alu.add)
                nc.vector.tensor_tensor(out=q_sb[:, lo:hi], in0=q_sb[:, lo:hi],
                                        in1=sc_sb[:, lo:hi], op=alu.max)
                nc.scalar.activation(out=ex_sb[:, lo:hi], in_=q_sb[:, lo:hi],
                                     func=act.Exp)

            def build_m(s):
                _, t0, t1 = sb_rng(s)
                T_s = t1 - t0
                # M[p, u*T_s + t] = ex[p,t] * (u == rr[p,t]); packed fp16
                # last dims everywhere -> DVE 2x_1p
                M = mp.tile([128, BW * TMAX], f16, tag="M", name="M")
                M3 = M[:, :BW * T_s].rearrange("p (u t) -> p u t", u=BW)
                io3 = iota_rep[:].rearrange("p (u t) -> p u t", u=BW)[:, :, :T_s]
                rr3 = rr_sb[:, t0:t1].rearrange("p (o t) -> p o t", o=1) \
                                     .to_broadcast([128, BW, T_s])
                ex3 = ex_sb[:, t0:t1].rearrange("p (o t) -> p o t", o=1) \
                                     .to_broadcast([128, BW, T_s])
                nc.vector.tensor_tensor(out=M3, in0=io3, in1=rr3, op=alu.is_equal)
                nc.vector.tensor_tensor(out=M3, in0=M3, in1=ex3, op=alu.mult)
                return M

            # G and M both pipelined two superblocks ahead: out-DMA triggers
            # (gated on division) never head-of-line-block the next G load,
            # and each superblock's division+PSUM-release runs on DVE well
            # before the matmuls that reuse those PSUM banks
            Gq = [G0, load_g(1)]
            Mq = [build_m(0), build_m(1)]
            for s in range(nsb):
                j0, t0, t1 = sb_rng(s)
                T_s = t1 - t0
                eng_o = nc.scalar if s % 2 == 0 else nc.sync
                G = Gq.pop(0)
                M = Mq.pop(0)
                if s + 2 < nsb:
                    Gq.append(load_g(s + 2))
                    Mq.append(build_m(s + 2))

                pgs = []
                for g, gsz in enumerate(PGRP):
                    pgs.append(pools[g].tile([BW, gsz * W65], f32, space="PSUM",
                                             tag=f"pg{g}", name=f"pg{g}"))
                jloc = 0
                for g, gsz in enumerate(PGRP):
                    for b in range(gsz):
                        j = j0 + jloc
                        tj = int(Tj[j])
                        tb = int(base[j]) - t0
                        for t in range(tj):
                            rel = tb + t
                            nc.tensor.matmul(
                                out=pgs[g][:, b * W65:(b + 1) * W65],
                                lhsT=M[:, rel:rel + (BW - 1) * T_s + 1:T_s],
                                rhs=G[:, rel * W65:(rel + 1) * W65],
                                start=(t == 0), stop=(t == tj - 1))
                        jloc += 1

                # batched denominators (eps folded into the gather):
                # dn[u, jloc] <- pg[:, 64::65] + 1e-10
                dn = fp.tile([BW, SBB], f32, tag="dn")
                o = 0
                for g, gsz in enumerate(PGRP):
                    nc.vector.tensor_scalar(out=dn[:, o:o + gsz],
                                            in0=pgs[g][:, OUT_DIM::W65],
                                            scalar1=1e-10, scalar2=None,
                                            op0=alu.add)
                    o += gsz
                dinv = fp.tile([BW, SBB], f32, tag="di")
                nc.vector.reciprocal(out=dinv[:], in_=dn[:])

                out_stage = op.tile([BW, SBB * OUT_DIM], f16, tag="ost")
                # ACT: scaled per-block copies for groups 0,1 minus a tail
                # that DVE handles as batched broadcast-mults (engine balance);
                # the final superblock goes all-DVE so the post-matmul tail is
                # short (nothing overlaps it anyway)
                tail = PGRP[1] if s == nsb - 1 else DVE_TAIL
                dve_parts = [(1, PGRP[1] - tail, tail), (2, 0, PGRP[2])]
                if s == nsb - 1:
                    dve_parts.insert(0, (0, 0, PGRP[0]))
                jloc = 0
                for g, gsz in enumerate(PGRP[:2]):
                    for b in range(gsz):
                        if s == nsb - 1 or (g == 1 and b >= gsz - tail):
                            break
                        nc.scalar.activation(
                            out=out_stage[:, jloc * OUT_DIM:(jloc + 1) * OUT_DIM],
                            in_=pgs[g][:, b * W65:b * W65 + OUT_DIM],
                            func=act.Copy, scale=dinv[:, jloc:jloc + 1])
                        jloc += 1
                for g, b0, bn in dve_parts:
                    if bn == 0:
                        continue
                    jb = b0 + sum(PGRP[:g])
                    nc.vector.tensor_tensor(
                        out=out_stage[:, jb * OUT_DIM:(jb + bn) * OUT_DIM]
                            .rearrange("p (b f) -> p b f", b=bn),
                        in0=pgs[g][:, b0 * W65:(b0 + bn) * W65]
                            .rearrange("p (b f) -> p b f", b=bn)[:, :, :OUT_DIM],
                        in1=dinv[:, jb:jb + bn].rearrange(
                            "p (b o) -> p b o", o=1).to_broadcast([BW, bn, OUT_DIM]),
                        op=alu.mult)

                out_ap = d_out[j0 * BW:(j0 + SBB) * BW, :].rearrange(
                    "(b p) f -> p b f", p=BW)
                in_ap = out_stage[:].rearrange("p (b f) -> p b f", b=SBB)
                eng_o.dma_start(out=out_ap, in_=in_ap)
    nc.compile()
    return nc


def _block_bounds(row):
    """DP over node boundaries: contiguous blocks of <= BW nodes minimizing
    sum of ceil(cnt/128) + DP_LAM per block (128-aligned edge counts)."""
    deg = np.bincount(row, minlength=N_NODES)
    pre = np.zeros(N_NODES + 1, np.int64)
    pre[1:] = np.cumsum(deg)
    prel = pre.tolist()
    INF = float("inf")
    f = [INF] * (N_NODES + 1)
    f[0] = 0.0
    choice = [0] * (N_NODES + 1)
    for n in range(1, N_NODES + 1):
        best = INF
        bk = 1
        pn = prel[n]
        for m in range(max(0, n - BW), n):
            c = f[m] + (pn - prel[m] + 127) // 128 + DP_LAM
            if c < best:
                best = c
                bk = n - m
        f[n] = best
        choice[n] = bk
    bounds = [N_NODES]
    n = N_NODES
    while n > 0:
        n -= choice[n]
        bounds.append(n)
    return np.array(bounds[::-1], np.int64), pre


def _prep_structure(row, col):
    """Variable-size dest-node blocks (<=32 nodes, ~128-aligned edge counts);
    permute blocks onto (core, slot) pairs so that blocks sharing a slot
    have similar edge counts; assign each edge a (partition p, tile t)."""
    bounds, pre = _block_bounds(row)
    nb = len(bounds) - 1
    cnt_real = pre[bounds[1:]] - pre[bounds[:-1]]
    bpc = -(-nb // CORES)
    bpc = -(-bpc // SBB) * SBB          # pad to a multiple of SBB
    NGB = CORES * bpc
    cnt = np.zeros(NGB, np.int64)
    cnt[:nb] = cnt_real
    sorted_ids = np.argsort(-cnt, kind="stable")
    blk_core = np.empty(NGB, np.int64)
    blk_slot = np.empty(NGB, np.int64)
    k = np.arange(NGB)
    blk_core[sorted_ids] = k % CORES
    blk_slot[sorted_ids] = k // CORES
    Tj = np.maximum(1, (cnt[sorted_ids[::CORES]] + 127) // 128)
    base = np.zeros(bpc + 1, np.int64)
    base[1:] = np.cumsum(Tj)
    Ttot = int(base[-1])

    gb = np.searchsorted(bounds, row, side="right") - 1
    key = blk_core[gb] * bpc + blk_slot[gb]
    kcnt = np.bincount(key, minlength=NGB)
    order = np.argsort(key, kind="stable")
    starts = np.zeros(NGB, np.int64)
    starts[1:] = np.cumsum(kcnt)[:-1]
    rank = np.arange(N_EDGES, dtype=np.int64) - np.repeat(starts, kcnt)
    key_s = key[order]
    core_s = key_s // bpc
    slot_s = key_s - core_s * bpc
    t_loc = rank >> 7
    p_s = rank & 127
    tglob = base[slot_s] + t_loc
    return dict(order=order, core_s=core_s, p_s=p_s, tglob=tglob,
                rel_s=(row[order] - bounds[gb[order]]), Tj=Tj, base=base,
                Ttot=Ttot, sorted_ids=sorted_ids, bounds=bounds, nb=nb,
                bpc=bpc)


def _run_spmd(nc, in_maps, trace=False):
    from concourse import bass_utils
    res = bass_utils.run_bass_kernel_spmd(
        nc, in_maps, core_ids=list(range(CORES)), trace=trace)
    return res


def kernel(h, row, col, W, a):
    trace = bool(os.environ.get("GAT_TRACE"))
    if trace:
        try:
            import ntff_shim
            ntff_shim.install()
        except Exception:
            trace = False

    h = np.ascontiguousarray(np.asarray(h, dtype=np.float32))
    W = np.ascontiguousarray(np.asarray(W, dtype=np.float32))
    a = np.ascontiguousarray(np.asarray(a, dtype=np.float32)).reshape(2 * OUT_DIM)
    row = np.asarray(row).astype(np.int64)
    col = np.asarray(col).astype(np.int64)

    # ---- pass 1: [Wh | s_src | s_dst], node-sharded, fp16 ----
    nc1 = _build_pass1()
    waug = np.concatenate(
        [W, (W @ a[:OUT_DIM])[:, None], (W @ a[OUT_DIM:])[:, None]],
        axis=1).astype(np.float16)
    in_maps1 = []
    for c in range(CORES):
        hpad = np.zeros((NPP, IN_DIM), np.float16)
        hpad[:NPC] = h[c * NPC:(c + 1) * NPC]
        in_maps1.append({"hT": np.ascontiguousarray(hpad.T), "waug": waug})
    res1 = _run_spmd(nc1, in_maps1, trace=trace)
    if trace:
        LAST_STATS["pass1_ns"] = res1.exec_time_ns

    WhA = np.ones((N_NODES, W65), np.float16)
    s_src = np.empty(N_NODES, np.float32)
    s_dst = np.empty(N_NODES, np.float32)
    for c in range(CORES):
        whT = res1.results[c]["whT"]
        WhA[c * NPC:(c + 1) * NPC, :OUT_DIM] = whT[:OUT_DIM, :NPC].T
        s_src[c * NPC:(c + 1) * NPC] = whT[OUT_DIM, :NPC]
        s_dst[c * NPC:(c + 1) * NPC] = whT[OUT_DIM + 1, :NPC]

    # ---- host: edge-slot structure + replicated-Wh message streams ----
    st = _prep_structure(row, col)
    Tj, Ttot = st["Tj"], st["Ttot"]
    cs, ps, tg = st["core_s"], st["p_s"], st["tglob"]
    row_s = row[st["order"]]
    col_s = col[st["order"]]

    msg = np.zeros((CORES, 128, Ttot, W65), np.float16)
    msg[cs, ps, tg] = WhA[col_s]
    q_edge = s_src[row_s] + s_dst[col_s]
    emax = float(np.max(np.maximum(q_edge, NEG_SLOPE * q_edge)))
    q = np.full((CORES, 128, Ttot), PAD_Q, np.float32)
    q[cs, ps, tg] = q_edge - emax          # device leakyrelu expects q-emax
    rr = np.zeros((CORES, 128, Ttot), np.float16)
    rr[cs, ps, tg] = st["rel_s"].astype(np.float16)

    # ---- pass 2: attention + segment sum ----
    bpc = st["bpc"]
    nc2 = _build_pass2(Tj, Ttot, emax, bpc)
    base = st["base"]
    nsb = bpc // SBB
    TMAX = int(max(base[(s + 1) * SBB] - base[s * SBB] for s in range(nsb)))
    iota_np = np.broadcast_to(
        np.repeat(np.arange(BW, dtype=np.float16), TMAX)[None, :],
        (128, BW * TMAX))
    iota_np = np.ascontiguousarray(iota_np)
    in_maps2 = [{"msg": msg[c].reshape(128, Ttot * W65),
                 "q": q[c], "rr": rr[c], "iota": iota_np}
                for c in range(CORES)]
    res2 = _run_spmd(nc2, in_maps2, trace=trace)
    if trace:
        LAST_STATS["pass2_ns"] = res2.exec_time_ns
        LAST_STATS["total_ns"] = (res1.exec_time_ns or 0) + (res2.exec_time_ns or 0)

    out = np.empty((N_NODES, OUT_DIM), np.float32)
    sorted_ids = st["sorted_ids"]
    bounds, nb = st["bounds"], st["nb"]
    for c in range(CORES):
        dev = res2.results[c]["out"]
        for j in range(bpc):
            g = int(sorted_ids[j * CORES + c])
            if g >= nb:
                continue
            n0, n1 = int(bounds[g]), int(bounds[g + 1])
            out[n0:n1] = dev[j * BW:j * BW + (n1 - n0)]
    return out
